# revision 1
# baseline (speedup 1.0000x reference)
"""ConvChunk2d patch-extraction kernel for Trainium2 (8 NeuronCores).

Reference computes, for x of shape (8, 64, 128, 128):
    out[n, y*128 + xx, c, a, b] = xpad[n, (a*192 + b*64 + c) // 9, y + a, xx + b]
with xpad zero-padded by 1 on H/W, output shape (8*16384, 64, 3, 3).

Pure data movement (gather + replication), memory-bound.  Strategy:
data-parallel over batch (1 image per core).  Per core:
  - Load input as A0[y_partition, ch, x+1] (x zero-padded in the free dim),
    plus partition-shifted copies Am (row y-1) / Ap (row y+1) loaded
    directly from HBM (compute engines are partition-lockstep, so the
    kernel row-shift must be materialized in SBUF).
  - For p = 3a+b and s in [0,9): the output columns j = c*9 + p with
    c = 9*ch + s - 64p form an affine family over ch, so one strided
    tensor_copy per (p, s) moves all of them (81 copies per x-block),
    spread across Vector/Scalar/GPSIMD engines.
  - Output tiles (128 rows y, XB*576 floats) DMA out as large contiguous
    runs per partition.
"""

import math

import numpy as np

import concourse.bacc as bacc
import concourse.bass as bass
import concourse.mybir as mybir
from concourse.bass_utils import run_bass_kernel_spmd
from concourse.tile import TileContext

N, C, H, W = 8, 64, 128, 128
K = 3
L = H * W
J = C * K * K  # 576 output columns per spatial location
XB = 16  # x-block width; out tile = [128, XB*J] floats
NBLK = W // XB
F32 = mybir.dt.float32


def _jobs():
    """(a, b, ch_lo, cnt, c0, p) for each affine copy family."""
    jobs = []
    for p in range(K * K):
        a, b = divmod(p, K)
        for s in range(9):
            ch_lo = math.ceil((64 * p - s) / 9)
            ch_hi = (63 + 64 * p - s) // 9
            cnt = ch_hi - ch_lo + 1
            c0 = 9 * ch_lo + s - 64 * p
            jobs.append((a, b, ch_lo, cnt, c0, p))
    return jobs


def build_nc():
    nc = bacc.Bacc("TRN2")
    x = nc.declare_dram_parameter("x", [C, H, W], F32, isOutput=False)
    out = nc.declare_dram_parameter("out", [L, J], F32, isOutput=True)

    with TileContext(nc) as tc:
        with (
            tc.tile_pool(name="a", bufs=1) as apool,
            tc.tile_pool(name="t", bufs=2) as tpool,
        ):
            A0 = apool.tile([128, C, W + 2], F32, tag="a0")
            Am = apool.tile([128, C, W + 2], F32, tag="am")
            Ap = apool.tile([128, C, W + 2], F32, tag="ap")
            zrow = apool.tile([128, 65], F32, tag="zrow")
            nc.vector.memset(zrow[:, :], 0.0)

            # Zero-pad columns x=0 and x=W+1 of all three tiles.
            for Ak in (A0, Am, Ap):
                nc.vector.memset(Ak[:, :, 0:1], 0.0)
                nc.vector.memset(Ak[:, :, W + 1 : W + 2], 0.0)
            # Load x[ch, y, xx] -> A0[y, ch, xx+1], plus partition-shifted
            # copies Am[y] = row y-1, Ap[y] = row y+1, straight from HBM.
            # Constraints discovered on HW:
            #  - Only 2D-AP, FULL-128-partition HWDGE DMAs get split across
            #    the 16 SDMA engines; 3D APs or 127-partition dests
            #    serialize onto engine 0 (~25 ns/descriptor, one engine).
            #  - So the shifted loads wrap the HBM source by one row into
            #    the neighboring channel (flat view) to keep 128 partitions;
            #    the garbage row lands in a partition that is zeroed after.
            nc.sync.dma_start(
                out=A0[:, :, 1 : W + 1], in_=x[:, :, :].transpose([1, 0, 2])
            )
            # Bulk shifted loads keep 128 partitions (anything else
            # serializes onto one SDMA engine) by wrapping the flat HBM row
            # index by one row: partition 0 of Am (and 127 of Ap) receives a
            # garbage row from the adjacent channel, fixed up below.  The
            # channel at the tensor edge (ch=0 for Am, ch=63 for Ap) cannot
            # wrap in-bounds, so it loads as a small 127-partition DMA.
            sm = x[1:C, :, :].transpose([1, 0, 2])
            sm.offset -= W  # (y, ch, xx) -> row y-1 of ch, for ch in [1, C)
            nc.sync.dma_start(out=Am[:, 1:C, 1 : W + 1], in_=sm)
            nc.sync.dma_start(out=Am[1:128, 0, 1 : W + 1], in_=x[0, 0 : H - 1, :])
            sp = x[0 : C - 1, :, :].transpose([1, 0, 2])
            sp.offset += W  # row y+1 of ch, for ch in [0, C-1)
            nc.sync.dma_start(out=Ap[:, 0 : C - 1, 1 : W + 1], in_=sp)
            nc.sync.dma_start(out=Ap[0:127, C - 1, 1 : W + 1], in_=x[C - 1, 1:H, :])
            # Boundary rows must read as zero padding: Am[0] = Ap[127] = 0
            # (rows -1 and 128 of the image).  Compute-engine partition
            # bases must be quadrant-aligned, so they can't be memset in
            # place; DMA zeros into them after the wrapped loads.
            nc.sync.dma_start(out=Am[0:1, :, :], in_=zrow[:, :])
            nc.sync.dma_start(out=Ap[127:128, :, :], in_=zrow[:, :])

            jobs = _jobs()
            outr = out[:, :].rearrange("(y xx) j -> y xx j", xx=W)
            # Greedy engine balancing with measured per-copy cost models (ns):
            # DVE ~ 75 + (58+e)/0.96, ACT ~ (224+e)/1.2, GPSIMD ~ 360 + 1.22e.
            load = [0.0, 0.0, 0.0]
            for blk in range(NBLK):
                x0 = blk * XB
                T = tpool.tile([128, XB, C, K * K], F32, tag="t")
                for a, b, ch_lo, cnt, c0, p in jobs:
                    Ak = (Am, A0, Ap)[a]
                    dst = T[:, :, c0 : c0 + 9 * (cnt - 1) + 1 : 9, p].transpose(
                        [0, 2, 1]
                    )
                    src = Ak[:, ch_lo : ch_lo + cnt, x0 + b : x0 + b + XB]
                    e = cnt * XB
                    costs = (75 + (58 + e) / 0.96, (224 + e) / 1.2, 360 + 1.22 * e)
                    eng = min(range(3), key=lambda i: load[i] + costs[i])
                    load[eng] += costs[eng]
                    if eng == 0:
                        nc.vector.tensor_copy(dst, src)
                    elif eng == 1:
                        nc.scalar.copy(dst, src)
                    else:
                        nc.gpsimd.tensor_copy(dst, src)
                nc.sync.dma_start(
                    out=outr[:, x0 : x0 + XB, :],
                    in_=T[:, :, :, :].rearrange("pp xx c q -> pp xx (c q)"),
                )
    nc.finalize()
    return nc


def kernel(x):
    x = np.ascontiguousarray(np.asarray(x, dtype=np.float32))
    assert x.shape == (N, C, H, W), x.shape
    nc = build_nc()
    in_maps = [{"x": x[n]} for n in range(N)]
    res = run_bass_kernel_spmd(nc, in_maps, list(range(N)))
    outs = [np.asarray(res.results[i]["out"]).reshape(L, C, K, K) for i in range(N)]
    return np.concatenate(outs, axis=0)



# revision 4
# speedup vs baseline: 1.1717x; 1.1717x over previous
"""ConvChunk2d patch-extraction kernel for Trainium2 (8 NeuronCores).

Reference computes, for x of shape (8, 64, 128, 128):
    out[n, y*128 + xx, c, a, b] = xpad[n, (a*192 + b*64 + c) // 9, y + a, xx + b]
with xpad zero-padded by 1 on H/W, output shape (8*16384, 64, 3, 3).

Pure data movement (gather + replication), memory-bound.  Strategy:
data-parallel over batch (1 image per core).  Per core:
  - Load input ONCE as A0[y_partition, ch, x+1] (x zero-padded in the free
    dim).  The row-shifted copies (row y-1 / y+1 in partition y) that the
    partition-lockstep compute engines need are produced on the idle
    TensorEngine: matmul with a 0/1 shift-permutation matrix (extra kernel
    input) into PSUM, then copied to small per-block SBUF halo tiles.
    This removes the 2 extra full-image HBM loads (the baseline's Am/Ap
    loads were 16k extra 512B DMA packets, ~35 us of serial input phase).
  - For p = 3a+b and s in [0,9): the output columns j = c*9 + p with
    c = 9*ch + s - 64p form an affine family over ch, so one strided
    tensor_copy per (p, s) moves all of them (81 copies per x-block),
    spread across Vector/Scalar/GPSIMD engines.  x-blocks of 30 (vs 16)
    halve the per-copy instruction overhead; a small first block (8)
    lets the output DMA start early.
  - Output tiles (128 rows y, XB*576 floats) DMA out as large contiguous
    runs per partition (XB*2304 bytes).
"""

import math

import numpy as np

import concourse.bacc as bacc
import concourse.bass as bass
import concourse.mybir as mybir
from concourse.bass_utils import run_bass_kernel_spmd
from concourse.tile import TileContext

N, C, H, W = 8, 64, 128, 128
K = 3
L = H * W
J = C * K * K  # 576 output columns per spatial location
BLOCKS = [(0, 8), (8, 30), (38, 30), (68, 30), (98, 30)]  # (x0, xb)
XBMAX = 30
F32 = mybir.dt.float32
F32R = mybir.dt.float32r


def _jobs():
    """(a, b, ch_lo, cnt, c0, p) for each affine copy family."""
    jobs = []
    for p in range(K * K):
        a, b = divmod(p, K)
        for s in range(9):
            ch_lo = math.ceil((64 * p - s) / 9)
            ch_hi = (63 + 64 * p - s) // 9
            cnt = ch_hi - ch_lo + 1
            c0 = 9 * ch_lo + s - 64 * p
            jobs.append((a, b, ch_lo, cnt, c0, p))
    return jobs


def build_nc():
    nc = bacc.Bacc("TRN2")
    x = nc.declare_dram_parameter("x", [C, H, W], F32, isOutput=False)
    sh = nc.declare_dram_parameter("sh", [128, 256], F32, isOutput=False)
    out = nc.declare_dram_parameter("out", [L, J], F32, isOutput=True)

    with TileContext(nc) as tc:
        with (
            tc.tile_pool(name="a", bufs=1) as apool,
            tc.tile_pool(name="h", bufs=2) as hpool,
            tc.tile_pool(name="t", bufs=2) as tpool,
            tc.tile_pool(name="ps", bufs=8, space="PSUM") as pspool,
        ):
            A0 = apool.tile([128, C, W + 2], F32, tag="a0")
            SH = apool.tile([128, 256], F32, tag="sh")
            nc.sync.dma_start(out=SH[:, :], in_=sh[:, :])

            # Zero-pad columns x=0 and x=W+1.
            nc.vector.memset(A0[:, :, 0:1], 0.0)
            nc.vector.memset(A0[:, :, W + 1 : W + 2], 0.0)
            # Load x[ch, y, xx] -> A0[y, ch, xx+1], split in two ch-halves so
            # the first shift-matmuls can start under the load tail.
            nc.sync.dma_start(
                out=A0[:, 0 : C // 2, 1 : W + 1],
                in_=x[0 : C // 2, :, :].transpose([1, 0, 2]),
            )
            nc.sync.dma_start(
                out=A0[:, C // 2 : C, 1 : W + 1],
                in_=x[C // 2 : C, :, :].transpose([1, 0, 2]),
            )

            jobs = _jobs()
            outr = out[:, :].rearrange("(y xx) j -> y xx j", xx=W)
            # Greedy engine balancing with measured per-copy cost models (ns):
            # DVE ~ 75 + (58+e)/0.96, ACT ~ (224+e)/1.2, GPSIMD ~ 360 + 1.22e.
            load = [0.0, 0.0, 0.0]
            for x0, xb in BLOCKS:
                hw = xb + 2  # halo width in padded-x columns [x0, x0+hw)
                T = tpool.tile([128, XBMAX, C, K * K], F32, tag="t")
                Hm = hpool.tile([128, C, XBMAX + 2], F32, tag="hm")
                Hp = hpool.tile([128, C, XBMAX + 2], F32, tag="hp")
                # Row-shifted halos via TensorE: psum[m, :] = sum_k S[k, m] A0[k, :]
                # in 16-channel chunks (<=512 f32 = 1 PSUM bank each).
                for hi, (Hk, s0) in enumerate(((Hm, 0), (Hp, 128))):
                    for ci in range(4):
                        ch0 = ci * 16
                        P = pspool.tile([128, 16 * (XBMAX + 2)], F32, tag="ps")
                        pc = P[:, : 16 * hw].rearrange("y (c w) -> y c w", c=16)
                        nc.tensor.matmul(
                            pc,
                            SH[:, s0 : s0 + 128],
                            A0[:, ch0 : ch0 + 16, x0 : x0 + hw],
                        )
                        e = 16 * hw
                        if (hi * 4 + ci) % 2 == 0:
                            nc.vector.tensor_copy(Hk[:, ch0 : ch0 + 16, 0:hw], pc)
                            load[0] += 75 + (58 + e) / 0.96
                        else:
                            nc.scalar.copy(Hk[:, ch0 : ch0 + 16, 0:hw], pc)
                            load[1] += (224 + e) / 1.2
                for a, b, ch_lo, cnt, c0, p in jobs:
                    dst = T[:, :xb, c0 : c0 + 9 * (cnt - 1) + 1 : 9, p].transpose(
                        [0, 2, 1]
                    )
                    if a == 1:
                        src = A0[:, ch_lo : ch_lo + cnt, x0 + b : x0 + b + xb]
                    else:
                        Hk = Hm if a == 0 else Hp
                        src = Hk[:, ch_lo : ch_lo + cnt, b : b + xb]
                    e = cnt * xb
                    costs = (75 + (58 + e) / 0.96, (224 + e) / 1.2, 360 + 1.22 * e)
                    eng = min(range(3), key=lambda i: load[i] + costs[i])
                    load[eng] += costs[eng]
                    if eng == 0:
                        nc.vector.tensor_copy(dst, src)
                    elif eng == 1:
                        nc.scalar.copy(dst, src)
                    else:
                        nc.gpsimd.tensor_copy(dst, src)
                nc.sync.dma_start(
                    out=outr[:, x0 : x0 + xb, :],
                    in_=T[:, :xb, :, :].rearrange("pp xx c q -> pp xx (c q)"),
                )
    nc.finalize()
    return nc


def _shift_mats():
    s = np.zeros((128, 256), dtype=np.float32)
    s[:, 0:128] = np.eye(128, k=1, dtype=np.float32)  # S_m: out[y] = in[y-1]
    s[:, 128:256] = np.eye(128, k=-1, dtype=np.float32)  # S_p: out[y] = in[y+1]
    return s


def make_in_maps(x):
    s = _shift_mats()
    return [{"x": x[n], "sh": s} for n in range(N)]


def kernel(x):
    x = np.ascontiguousarray(np.asarray(x, dtype=np.float32))
    assert x.shape == (N, C, H, W), x.shape
    nc = build_nc()
    in_maps = make_in_maps(x)
    res = run_bass_kernel_spmd(nc, in_maps, list(range(N)))
    outs = [np.asarray(res.results[i]["out"]).reshape(L, C, K, K) for i in range(N)]
    return np.concatenate(outs, axis=0)


# revision 7
# speedup vs baseline: 1.1934x; 1.0186x over previous
"""ConvChunk2d patch-extraction kernel for Trainium2 (8 NeuronCores).

Reference computes, for x of shape (8, 64, 128, 128):
    out[n, y*128 + xx, c, a, b] = xpad[n, (a*192 + b*64 + c) // 9, y + a, xx + b]
with xpad zero-padded by 1 on H/W, output shape (8*16384, 64, 3, 3).

Pure data movement (gather + replication), memory-bound.  Strategy:
data-parallel over batch (1 image per core).  Per core:
  - Load input ONCE as A0[y_partition, ch, x+1] (x zero-padded in the free
    dim).  The row-shifted copies (row y-1 / y+1 in partition y) that the
    partition-lockstep compute engines need are produced on the idle
    TensorEngine: matmul with a 0/1 shift-permutation matrix (extra kernel
    input) into PSUM, then copied to small per-block SBUF halo tiles.
  - Output assembly: out column j = c*9 + 3a+b reads channel
    ch(a,b,c) = (192a + 64b + c)//9.  For fixed a and phase
    phi = (192a + c) % 9, the (c, b) pairs form an affine lattice:
    c = c0 + 9t, ch = ch0 + t + 7b (valid for phi <= 6), so ONE strided
    tensor_copy moves cnt*3*xb elements (custom AP with a 7*pitch+1
    stride for the b axis); phi in {7, 8} fall back to per-b copies.
    39 copies per x-block instead of 81, spread across Vector/Scalar/
    GPSIMD by measured cost models.
  - Output tiles (128 rows y, xb*576 floats) DMA out as large contiguous
    runs per partition (xb*2304 bytes).  Small first/last blocks shorten
    the pipeline ramp and drain.
"""

import numpy as np

import concourse.bacc as bacc
import concourse.bass as bass
import concourse.mybir as mybir
from concourse.bass_utils import run_bass_kernel_spmd
from concourse.tile import TileContext

N, C, H, W = 8, 64, 128, 128
K = 3
L = H * W
J = C * K * K  # 576 output columns per spatial location
BLOCKS = [(0, 4), (4, 30), (34, 30), (64, 30), (94, 30), (124, 4)]
XBMAX = 30
F32 = mybir.dt.float32
F32R = mybir.dt.float32r


def _jobs2():
    """Merged copy families.

    ("m", a, c0, cnt, ch0): dst T[:, :, c0+9t, 3a+b] <- src[ch0 + t + 7b]
        for t in [0,cnt), b in [0,3)  (one copy, custom b-stride AP)
    ("s", a, b, c0, cnt, ch): dst T[:, :, c0+9t, 3a+b] <- src[ch + t]
    """
    jobs = []
    for a in range(3):
        for phi in range(9):
            c0 = (phi - 192 * a) % 9
            cnt = (64 - c0 + 8) // 9
            ch0 = (192 * a + c0) // 9
            if phi <= 6:
                jobs.append(("m", a, c0, cnt, ch0))
            else:
                for b in range(3):
                    off = (phi + 64 * b) // 9
                    jobs.append(("s", a, b, c0, cnt, ch0 + off))
    return jobs


def build_nc():
    nc = bacc.Bacc("TRN2")
    x = nc.declare_dram_parameter("x", [C, H, W], F32, isOutput=False)
    sh = nc.declare_dram_parameter("sh", [128, 256], F32, isOutput=False)
    out = nc.declare_dram_parameter("out", [L, J], F32, isOutput=True)

    with TileContext(nc) as tc:
        with (
            tc.tile_pool(name="a", bufs=1) as apool,
            tc.tile_pool(name="h", bufs=2) as hpool,
            tc.tile_pool(name="t", bufs=2) as tpool,
            tc.tile_pool(name="ps", bufs=8, space="PSUM") as pspool,
        ):
            A0 = apool.tile([128, C, W + 2], F32, tag="a0")
            SH = apool.tile([128, 256], F32, tag="sh")
            nc.sync.dma_start(out=SH[:, :], in_=sh[:, :])

            # Zero-pad columns x=0 and x=W+1.
            nc.vector.memset(A0[:, :, 0:1], 0.0)
            nc.vector.memset(A0[:, :, W + 1 : W + 2], 0.0)
            # Load x[ch, y, xx] -> A0[y, ch, xx+1], split in two ch-halves so
            # the first shift-matmuls can start under the load tail.
            nc.sync.dma_start(
                out=A0[:, 0 : C // 2, 1 : W + 1],
                in_=x[0 : C // 2, :, :].transpose([1, 0, 2]),
            )
            nc.sync.dma_start(
                out=A0[:, C // 2 : C, 1 : W + 1],
                in_=x[C // 2 : C, :, :].transpose([1, 0, 2]),
            )

            jobs = _jobs2()
            outr = out[:, :].rearrange("(y xx) j -> y xx j", xx=W)
            # Greedy engine balancing with trace-measured per-copy cost
            # models (ns, e = elements per partition):
            #   Vector  ~  95 + 3.15 e   (scattered-4B-write pattern)
            #   Scalar  ~ 289 + 1.61 e
            #   GpSimd  ~ 205 + 3.15 e
            # Merged copies write 12B runs; assume partial relief on V/G.
            load = [0.0, 0.0, 0.0]
            for x0, xb in BLOCKS:
                hw = xb + 2  # halo width in padded-x columns [x0, x0+hw)
                T = tpool.tile([128, XBMAX, C, K * K], F32, tag="t")
                Hm = hpool.tile([128, C, XBMAX + 2], F32, tag="hm")
                Hp = hpool.tile([128, C, XBMAX + 2], F32, tag="hp")
                # Row-shifted halos via TensorE: psum[m,:] = sum_k S[k,m] A0[k,:]
                # in 16-channel chunks (<=512 f32 = 1 PSUM bank each).
                for hi, (Hk, s0) in enumerate(((Hm, 0), (Hp, 128))):
                    for ci in range(4):
                        ch0 = ci * 16
                        P = pspool.tile([128, 16 * (XBMAX + 2)], F32, tag="ps")
                        pc = P[:, : 16 * hw].rearrange("y (c w) -> y c w", c=16)
                        nc.tensor.matmul(
                            pc,
                            SH[:, s0 : s0 + 128],
                            A0[:, ch0 : ch0 + 16, x0 : x0 + hw],
                        )
                        e = 16 * hw
                        if (hi * 4 + ci) % 2 == 0:
                            nc.vector.tensor_copy(Hk[:, ch0 : ch0 + 16, 0:hw], pc)
                            load[0] += 143 + 1.04 * e
                        else:
                            nc.scalar.copy(Hk[:, ch0 : ch0 + 16, 0:hw], pc)
                            load[1] += 230 + 0.89 * e
                for job in jobs:
                    a = job[1]
                    if a == 1:
                        Sk, pitch, xc = A0, W + 2, x0
                    else:
                        Sk, pitch, xc = (Hm if a == 0 else Hp), XBMAX + 2, 0
                    if job[0] == "m":
                        _, a, c0, cnt, ch0 = job
                        dst = T[:, :xb, c0 : c0 + 9 * (cnt - 1) + 1 : 9, 3 * a : 3 * a + 3]
                        src = (
                            Sk[:, ch0 : ch0 + cnt, xc : xc + xb]
                            .transpose([0, 2, 1])
                            .unsqueeze(3)
                        )
                        src.ap[3] = [7 * pitch + 1, 3]
                        e = 3 * cnt * xb
                        costs = (95 + 2.2 * e, 289 + 1.35 * e, 205 + 2.3 * e)
                    else:
                        _, a, b, c0, cnt, ch = job
                        q = 3 * a + b
                        dst = T[:, :xb, c0 : c0 + 9 * (cnt - 1) + 1 : 9, q]
                        src = Sk[:, ch : ch + cnt, xc + b : xc + b + xb].transpose(
                            [0, 2, 1]
                        )
                        e = cnt * xb
                        costs = (95 + 3.15 * e, 289 + 1.61 * e, 205 + 3.15 * e)
                    eng = min(range(3), key=lambda i: load[i] + costs[i])
                    load[eng] += costs[eng]
                    if eng == 0:
                        nc.vector.tensor_copy(dst, src)
                    elif eng == 1:
                        nc.scalar.copy(dst, src)
                    else:
                        nc.gpsimd.tensor_copy(dst, src)
                nc.sync.dma_start(
                    out=outr[:, x0 : x0 + xb, :],
                    in_=T[:, :xb, :, :].rearrange("pp xx c q -> pp xx (c q)"),
                )
    nc.finalize()
    return nc


def _shift_mats():
    s = np.zeros((128, 256), dtype=np.float32)
    s[:, 0:128] = np.eye(128, k=1, dtype=np.float32)  # S_m: out[y] = in[y-1]
    s[:, 128:256] = np.eye(128, k=-1, dtype=np.float32)  # S_p: out[y] = in[y+1]
    return s


def make_in_maps(x):
    s = _shift_mats()
    return [{"x": x[n], "sh": s} for n in range(N)]


def kernel(x):
    x = np.ascontiguousarray(np.asarray(x, dtype=np.float32))
    assert x.shape == (N, C, H, W), x.shape
    nc = build_nc()
    in_maps = make_in_maps(x)
    res = run_bass_kernel_spmd(nc, in_maps, list(range(N)))
    outs = [np.asarray(res.results[i]["out"]).reshape(L, C, K, K) for i in range(N)]
    return np.concatenate(outs, axis=0)


# revision 12
# speedup vs baseline: 1.2241x; 1.0257x over previous
"""ConvChunk2d patch-extraction kernel for Trainium2 (8 NeuronCores).

Reference computes, for x of shape (8, 64, 128, 128):
    out[n, y*128 + xx, c, a, b] = xpad[n, (a*192 + b*64 + c) // 9, y + a, xx + b]
with xpad zero-padded by 1 on H/W, output shape (8*16384, 64, 3, 3).

Pure data movement (gather + replication), memory-bound.  Strategy:
data-parallel over batch (1 image per core).  Per core:
  - Load input ONCE as A0[y_partition, ch, x+1] (x zero-padded in the free
    dim).  The row-shifted copies (row y-1 / y+1 in partition y) that the
    partition-lockstep compute engines need are produced on the idle
    TensorEngine: matmul with a 0/1 shift-permutation matrix (extra kernel
    input) into PSUM, then copied to small per-block SBUF halo tiles.
  - Output assembly: out column j = c*9 + 3a+b reads channel
    ch(a,b,c) = (192a + 64b + c)//9.  For fixed a and phase
    phi = (192a + c) % 9, the (c, b) pairs form an affine lattice:
    c = c0 + 9t, ch = ch0 + t + 7b (valid for phi <= 6), so ONE strided
    tensor_copy moves cnt*3*xb elements (custom AP with a 7*pitch+1
    stride for the b axis); phi in {7, 8} fall back to per-b copies.
    39 copies per x-block instead of 81, spread across Vector/Scalar/
    GPSIMD by measured cost models.
  - Output tiles (128 rows y, xb*576 floats) DMA out as large contiguous
    runs per partition (xb*2304 bytes).  Small first/last blocks shorten
    the pipeline ramp and drain.
"""

import numpy as np

import concourse.bacc as bacc
import concourse.bass as bass
import concourse.mybir as mybir
from concourse.bass_utils import run_bass_kernel_spmd
from concourse.tile import TileContext

N, C, H, W = 8, 64, 128, 128
K = 3
L = H * W
J = C * K * K  # 576 output columns per spatial location
BLOCKS = [(0, 4), (4, 12), (16, 28), (44, 28), (72, 28), (100, 24), (124, 4)]
XBMAX = 28
F32 = mybir.dt.float32
F32R = mybir.dt.float32r


def _jobs2():
    """Merged copy families.

    ("m", a, c0, cnt, ch0): dst T[:, :, c0+9t, 3a+b] <- src[ch0 + t + 7b]
        for t in [0,cnt), b in [0,3)  (one copy, custom b-stride AP)
    ("s", a, b, c0, cnt, ch): dst T[:, :, c0+9t, 3a+b] <- src[ch + t]
    """
    jobs = []
    for a in range(3):
        for phi in range(9):
            c0 = (phi - 192 * a) % 9
            cnt = (64 - c0 + 8) // 9
            ch0 = (192 * a + c0) // 9
            if phi <= 6:
                jobs.append(("m", a, c0, cnt, ch0))
            else:
                for b in range(3):
                    off = (phi + 64 * b) // 9
                    jobs.append(("s", a, b, c0, cnt, ch0 + off))
    return jobs


def build_nc():
    nc = bacc.Bacc("TRN2")
    x = nc.declare_dram_parameter("x", [C, H, W], F32, isOutput=False)
    sh = nc.declare_dram_parameter("sh", [128, 256], F32, isOutput=False)
    out = nc.declare_dram_parameter("out", [L, J], F32, isOutput=True)

    with TileContext(nc) as tc:
        with (
            tc.tile_pool(name="a", bufs=1) as apool,
            tc.tile_pool(name="h", bufs=2) as hpool,
            tc.tile_pool(name="t", bufs=2) as tpool,
            tc.tile_pool(name="ps", bufs=8, space="PSUM") as pspool,
        ):
            A0 = apool.tile([128, C, W + 2], F32, tag="a0")
            SH = apool.tile([128, 256], F32, tag="sh")
            nc.sync.dma_start(out=SH[:, :], in_=sh[:, :])

            # Zero-pad columns x=0 and x=W+1.
            nc.vector.memset(A0[:, :, 0:1], 0.0)
            nc.vector.memset(A0[:, :, W + 1 : W + 2], 0.0)
            # Load x[ch, y, xx] -> A0[y, ch, xx+1], split in ch-quarters so
            # the first shift-matmuls and copies can start under the load.
            for cq in range(0, C, 16):
                nc.sync.dma_start(
                    out=A0[:, cq : cq + 16, 1 : W + 1],
                    in_=x[cq : cq + 16, :, :].transpose([1, 0, 2]),
                )

            jobs = _jobs2()
            outr = out[:, :].rearrange("(y xx) j -> y xx j", xx=W)
            # Greedy engine balancing with trace-measured per-copy cost
            # models (ns, e = elements per partition).  3-D single copies:
            #   V 95+3.15e / S 289+1.61e / G 205+3.15e; 4-D merged copies:
            #   V 601+1.80e / S 292+1.80e / G 190+3.71e (4-D APs carry a
            #   large fixed cost on DVE).  The greedy picks, per family,
            #   merged-on-one-engine vs split-into-3 by resulting makespan.
            load = [0.0, 0.0, 0.0]
            for x0, xb in BLOCKS:
                hw = xb + 2  # halo width in padded-x columns [x0, x0+hw)
                T = tpool.tile([128, XBMAX, C, K * K], F32, tag="t")
                Hm = hpool.tile([128, C, XBMAX + 2], F32, tag="hm")
                Hp = hpool.tile([128, C, XBMAX + 2], F32, tag="hp")
                # Row-shifted halos via TensorE: psum[m,:] = sum_k S[k,m] A0[k,:]
                # in 16-channel chunks (<=512 f32 = 1 PSUM bank each).
                for hi, (Hk, s0) in enumerate(((Hm, 0), (Hp, 128))):
                    for ci in range(4):
                        ch0 = ci * 16
                        P = pspool.tile([128, 16 * (XBMAX + 2)], F32, tag="ps")
                        pc = P[:, : 16 * hw].rearrange("y (c w) -> y c w", c=16)
                        nc.tensor.matmul(
                            pc,
                            SH[:, s0 : s0 + 128],
                            A0[:, ch0 : ch0 + 16, x0 : x0 + hw],
                        )
                        e = 16 * hw
                        cv, cs = 147 + 1.03 * e, 254 + 0.84 * e
                        if load[0] + cv <= load[1] + cs:
                            nc.vector.tensor_copy(Hk[:, ch0 : ch0 + 16, 0:hw], pc)
                            load[0] += cv
                        else:
                            nc.scalar.copy(Hk[:, ch0 : ch0 + 16, 0:hw], pc)
                            load[1] += cs
                engines = (nc.vector.tensor_copy, nc.scalar.copy, nc.gpsimd.tensor_copy)

                def emit(dst, src, costs):
                    eng = min(range(3), key=lambda i: load[i] + costs[i])
                    load[eng] += costs[eng]
                    engines[eng](dst, src)

                def single_aps(Sk, xc, a, b, c0, cnt, ch):
                    dst = T[:, :xb, c0 : c0 + 9 * (cnt - 1) + 1 : 9, 3 * a + b]
                    src = Sk[:, ch : ch + cnt, xc + b : xc + b + xb].transpose(
                        [0, 2, 1]
                    )
                    return dst, src

                for job in jobs:
                    a = job[1]
                    if a == 1:
                        Sk, pitch, xc = A0, W + 2, x0
                    else:
                        Sk, pitch, xc = (Hm if a == 0 else Hp), XBMAX + 2, 0
                    if job[0] == "m":
                        _, a, c0, cnt, ch0 = job
                        em, es = 3 * cnt * xb, cnt * xb
                        cm = (601 + 1.80 * em, 292 + 1.80 * em, 190 + 3.71 * em)
                        cs = (95 + 3.15 * es, 289 + 1.61 * es, 205 + 3.15 * es)
                        # merged on one engine vs three singles, by makespan
                        lm = list(load)
                        im = min(range(3), key=lambda i: lm[i] + cm[i])
                        lm[im] += cm[im]
                        ls = list(load)
                        for _b in range(3):
                            i = min(range(3), key=lambda j: ls[j] + cs[j])
                            ls[i] += cs[i]
                        if (max(lm), sum(lm)) <= (max(ls), sum(ls)):
                            dst = T[
                                :, :xb, c0 : c0 + 9 * (cnt - 1) + 1 : 9, 3 * a : 3 * a + 3
                            ]
                            src = (
                                Sk[:, ch0 : ch0 + cnt, xc : xc + xb]
                                .transpose([0, 2, 1])
                                .unsqueeze(3)
                            )
                            src.ap[3] = [7 * pitch + 1, 3]
                            load[im] += cm[im]
                            engines[im](dst, src)
                        else:
                            for b in range(3):
                                dst, src = single_aps(
                                    Sk, xc, a, b, c0, cnt, ch0 + 7 * b
                                )
                                emit(dst, src, cs)
                    else:
                        _, a, b, c0, cnt, ch = job
                        dst, src = single_aps(Sk, xc, a, b, c0, cnt, ch)
                        es = cnt * xb
                        emit(dst, src, (95 + 3.15 * es, 289 + 1.61 * es, 205 + 3.15 * es))
                nc.sync.dma_start(
                    out=outr[:, x0 : x0 + xb, :],
                    in_=T[:, :xb, :, :].rearrange("pp xx c q -> pp xx (c q)"),
                )
    nc.finalize()
    return nc


def _shift_mats():
    s = np.zeros((128, 256), dtype=np.float32)
    s[:, 0:128] = np.eye(128, k=1, dtype=np.float32)  # S_m: out[y] = in[y-1]
    s[:, 128:256] = np.eye(128, k=-1, dtype=np.float32)  # S_p: out[y] = in[y+1]
    return s


def make_in_maps(x):
    s = _shift_mats()
    return [{"x": x[n], "sh": s} for n in range(N)]


def kernel(x):
    x = np.ascontiguousarray(np.asarray(x, dtype=np.float32))
    assert x.shape == (N, C, H, W), x.shape
    nc = build_nc()
    in_maps = make_in_maps(x)
    res = run_bass_kernel_spmd(nc, in_maps, list(range(N)))
    outs = [np.asarray(res.results[i]["out"]).reshape(L, C, K, K) for i in range(N)]
    return np.concatenate(outs, axis=0)
